# revision 43
# baseline (speedup 1.0000x reference)
"""DinoV2 attention (B=8, S=1370, D=1024, H=16, Dh=64) on 8 trn2 NeuronCores.

Sharding: data parallel over batch — core b computes batch element b end to
end; weights are replicated; no collectives.

Cost-model-driven design (TimelineSim charges matmuls by output free-dim
rows only; Activation by free rows + fixed per-inst access penalty):
  - All projection matmuls use full 128-partition contraction (the out-proj
    is a single 128-deep matmul per tile, not two 64-deep tile_position
    halves, which would be charged double).
  - A software pipeline keeps the PE stream dense: while the Act-bound
    attention inner loop runs for pair hp, the PE stream is fed "filler"
    matmuls (Q/K projections for hp+1, V projection quads, out-projection
    accumulation for already-finished chunks) via an emission-order queue.
  - Out-projection accumulates two head pairs per PSUM tile, then folds the
    result into an SBUF accumulator via DVE adds; tiles are pushed as soon
    as their OT query-columns normalize, so the end-of-kernel tail is only
    the last chunk's tiles.
  - Host pre-swizzles x and the weights into partition-major bf16 layouts
    (fp32r only where precision matters: OT and Wo) so every DMA has
    contiguous >=512B runs per partition and arrives in the order the PE
    stream consumes it (256-column chunks for x, so the first V projection
    chain starts ~4us in while the rest of x still streams).
  - The kernel-final chunk normalizes via a small PE broadcast matmul
    (ones^T @ 1/Z) instead of the DRAM Z-bounce, and the final fold+store
    chains read/write the accumulator in place, keeping the drain tail to
    the DVE fold throughput.
Measured (per-core instruction-cost model): 372.1us vs 496.7us baseline;
hardware rel err 5.1e-3 (budget 2e-2). PE busy is ~355us, the cost-model
floor for this decomposition (charged matmul rows: QKVO projections
355.6k cycles + scores/PV 482.2k cycles); fp8 DoubleRow would halve the
score matmuls but inherently costs 4.4e-2 max-norm error (measured on the
reference) and is excluded by the 2e-2 gate.
"""

import numpy as np
from collections import deque
from contextlib import ExitStack

import concourse.bass as bass
import concourse.mybir as mybir
import concourse.tile as tile
from concourse.bass_utils import run_bass_kernel_spmd

B = 8
S = 1370
SPAD = 1536          # xT columns padded to 256-col DMA chunks (the pad
                     # columns are never read; 512B runs per partition dodge
                     # the <512B DMA descriptor penalty)
NXC = SPAD // 256    # 6 xT DMA chunks
D = 1024
H = 16
DH = 64
P = 128
KT = D // P          # 8 contraction tiles over D
NPAIR = H // 2       # 8 head pairs
NQUAD = 4            # V projection in 4-head (256-col) quads
NST = 11             # s-tiles over the real 1370 rows (last has 90)
FP = mybir.dt.float32
FPR = mybir.dt.float32r
BF = mybir.dt.bfloat16
AF = mybir.ActivationFunctionType

ST_SIZES = [min(P, S - i * P) for i in range(NST)]
# Q/K projection column chunks (padded space; 256-wide so each chunk only
# needs 2 xT s-tiles, letting projections start while xT still streams in)
PROJ_CHUNKS = [(0, 256), (256, 256), (512, 256), (768, 256), (1024, S - 1024)]
# attention query chunks (real queries only)
ATT_CHUNKS = [(0, 512), (512, 512), (1024, S - 1024)]
# s-tiles whose OT columns are final after each attention chunk
CHUNK_STS = [range(0, 4), range(4, 8), range(8, 11)]
SCALE = 1.0 / np.sqrt(DH)


def _legalize_syncs(nc):
    """Move excess sem waits onto injected NoOps.

    This walrus build encodes at most one wait (plus one update) per TPB
    instruction; Tile emits several. Engines execute their streams in
    order and the Tile schedule is a topological order of the dependency
    DAG, so hoisting waits onto preceding same-engine NoOps preserves
    progress (anything scheduled earlier can still complete) and
    correctness (the instruction still starts only after all its waits).
    """
    nid = 0
    for f in nc.m.functions:
        for blk in f.blocks:
            out = []
            for inst in blk.instructions:
                si = inst.sync_info
                if si is not None:
                    waits = list(si.on_wait)
                    ups = list(si.on_update)
                    if len(waits) > 1:
                        for w in waits[:-1]:
                            nop = mybir.InstNoOp(
                                name=f"I-syncfix-{nid}",
                                engine=inst.engine, ins=[], outs=[],
                                sync_info=mybir.SyncInfo(on_wait=[w],
                                                         on_update=[]))
                            nid += 1
                            nc.register_instruction(nop)
                            out.append(nop)
                        inst.sync_info = mybir.SyncInfo(on_wait=waits[-1:],
                                                        on_update=ups)
                out.append(inst)
            blk.instructions = out


def build_nc(repeat=1):
    nc = bass.Bass()
    # host-swizzled inputs (see _in_maps)
    xR = nc.declare_dram_parameter("xR", [P, NXC, KT, 256], BF, isOutput=False)
    WqR = nc.declare_dram_parameter("WqR", [P, NPAIR, KT, P], BF,
                                    isOutput=False)
    WkR = nc.declare_dram_parameter("WkR", [P, NPAIR, KT, P], BF,
                                    isOutput=False)
    WvR = nc.declare_dram_parameter("WvR", [P, NQUAD, KT, 256], BF,
                                    isOutput=False)
    WoR = nc.declare_dram_parameter("WoR", [P, NPAIR, D], FPR, isOutput=False)
    bq = nc.declare_dram_parameter("bq", [D], FP, isOutput=False)
    bk = nc.declare_dram_parameter("bk", [D], FP, isOutput=False)
    bv = nc.declare_dram_parameter("bv", [D], FP, isOutput=False)
    bo = nc.declare_dram_parameter("bo", [D], FP, isOutput=False)
    out = nc.declare_dram_parameter("out", [S, D], FP, isOutput=True)
    # bounce buffer for replicating softmax denominators across partitions
    # (SBUF APs can't partition-broadcast, DRAM APs can)
    zdram = nc.dram_tensor("zbounce", [NPAIR, 3, 1024], FP)

    def bcast(handle, rows):
        a = handle[:]
        return bass.AP(tensor=a.tensor, offset=a.offset, ap=[[0, rows], *a.ap])

    xR_ap = xR[:]
    wq_ap = WqR[:].rearrange("p hp kt i -> p (hp kt) i")
    wk_ap = WkR[:].rearrange("p hp kt i -> p (hp kt) i")
    wv_ap = WvR[:].rearrange("p q kt j -> p (q kt) j")
    wo_ap = WoR[:].rearrange("p hp c -> p (hp c)")

    with ExitStack() as ctx:
        tc = ctx.enter_context(tile.TileContext(nc))
        const = ctx.enter_context(tc.tile_pool(name="const", bufs=1))
        persist = ctx.enter_context(tc.tile_pool(name="persist", bufs=1))
        psum = ctx.enter_context(tc.tile_pool(name="psum", bufs=1,
                                              space="PSUM"))
        wp = ctx.enter_context(tc.tile_pool(name="wp", bufs=2))
        wvp = ctx.enter_context(tc.tile_pool(name="wvp", bufs=2))
        wop = ctx.enter_context(tc.tile_pool(name="wop", bufs=2))
        qkp = ctx.enter_context(tc.tile_pool(name="qkp", bufs=2))
        otp = ctx.enter_context(tc.tile_pool(name="otp", bufs=3))
        ep = ctx.enter_context(tc.tile_pool(name="ep", bufs=2))
        zp = ctx.enter_context(tc.tile_pool(name="zp", bufs=1))
        obp = ctx.enter_context(tc.tile_pool(name="obp", bufs=2))

        # persistent state
        xT_sb = persist.tile([P, KT, SPAD], BF)        # x^T, kt-major
        v_sb = persist.tile([P, NST, H, DH + 1], BF)    # V + ones column
        acc_sb = persist.tile([P, NST, D], FP)          # out-proj accumulator

        ones_sb = const.tile([P, DH], FP)
        nc.vector.memset(ones_sb[DH:DH + 1, :], 1.0)
        bq_sb = const.tile([P, NPAIR], FP)
        bk_sb = const.tile([P, NPAIR], FP)
        bv_bc = const.tile([P, D], FP)
        bo_bc = const.tile([P, D], FP)

        wv_tiles = {}
        wq_tiles, wk_tiles = {}, {}
        wo_tiles = {}
        qt_tiles, kt_tiles = {}, {}
        ot_tiles = {}

        def dma_wv(q):
            wv_tiles[q] = wvp.tile([P, KT, 256], BF, tag="wv", name=f"wv{q}")
            nc.sync.dma_start(out=wv_tiles[q],
                              in_=wv_ap[:, q * KT:(q + 1) * KT, :])

        def dma_wqk(hp):
            wq_tiles[hp] = wp.tile([P, KT, P], BF, tag="wq", name=f"wq{hp}")
            wk_tiles[hp] = wp.tile([P, KT, P], BF, tag="wk", name=f"wk{hp}")
            nc.sync.dma_start(out=wq_tiles[hp],
                              in_=wq_ap[:, hp * KT:(hp + 1) * KT, :])
            nc.sync.dma_start(out=wk_tiles[hp],
                              in_=wk_ap[:, hp * KT:(hp + 1) * KT, :])

        def dma_wo(hp):
            wo_tiles[hp] = wop.tile([P, D], FPR, tag="wo", name=f"wo{hp}")
            nc.sync.dma_start(out=wo_tiles[hp],
                              in_=wo_ap[:, hp * D:(hp + 1) * D])

        # ---- startup DMA stream (arrival order == emission order): the
        # first V chain needs only wv quad 0 kt 0-3 + xT s-tile 0, so those
        # go out first in small pieces
        wv_tiles[0] = wvp.tile([P, KT, 256], BF, tag="wv", name="wv0")
        nc.sync.dma_start(out=wv_tiles[0][:, 0:4, :],
                          in_=wv_ap[:, 0:4, :])
        nc.sync.dma_start(out=xT_sb[:, 0:4, 0:256], in_=xR_ap[:, 0, 0:4])
        nc.sync.dma_start(out=xT_sb[:, 4:8, 0:256], in_=xR_ap[:, 0, 4:8])
        nc.sync.dma_start(out=wv_tiles[0][:, 4:8, :],
                          in_=wv_ap[:, 4:8, :])
        nc.sync.dma_start(out=xT_sb[:, :, 256:512], in_=xR_ap[:, 1])
        nc.sync.dma_start(out=bv_bc, in_=bcast(bv, P))
        dma_wqk(0)
        nc.sync.dma_start(out=bq_sb,
                          in_=bq[:].rearrange("(hp p) -> p hp", p=P))
        nc.sync.dma_start(out=bk_sb,
                          in_=bk[:].rearrange("(hp p) -> p hp", p=P))
        for xc in range(2, NXC):
            nc.sync.dma_start(out=xT_sb[:, :, xc * 256:(xc + 1) * 256],
                              in_=xR_ap[:, xc])
        nc.sync.dma_start(out=bo_bc, in_=bcast(bo, P))
        dma_wv(1)
        dma_wqk(1)
        dma_wo(0)

        # ones columns for the fused softmax denominators (Z row of PV out)
        for st in range(NST):
            nc.vector.memset(v_sb[:, st, :, DH:DH + 1], 1.0)

        # ---- filler chain generators (one PE/DVE instruction per yield)
        def gen_vchain(quad, st):
            sw = ST_SIZES[st]
            wv_t = wv_tiles[quad]
            vps = psum.tile([P, 512], FP, tag="mm", bufs=2)
            for kt in range(KT):
                nc.tensor.matmul(vps[:sw, :256],
                                 lhsT=xT_sb[:, kt, st * P:st * P + sw],
                                 rhs=wv_t[:, kt, :],
                                 start=(kt == 0), stop=(kt == KT - 1))
                yield
            h0 = quad * 4
            c0 = quad * 256
            nc.vector.tensor_add(
                v_sb[:sw, st, h0:h0 + 4, 0:DH],
                vps[:sw, :256].rearrange("p (h d) -> p h d", h=4),
                bv_bc[:sw, c0:c0 + 256].rearrange("p (h d) -> p h d", h=4))
            yield

        def gen_qkchain(hp, which, q0, cw):
            if which == "q":
                w_t, dst, b_sb = wq_tiles[hp], qt_tiles[hp], bq_sb
            else:
                w_t, dst, b_sb = wk_tiles[hp], kt_tiles[hp], bk_sb
            ps = psum.tile([P, 512], FP, tag="mm", bufs=2)
            for kt in range(KT):
                nc.tensor.matmul(ps[:, :cw],
                                 lhsT=w_t[:, kt, :],
                                 rhs=xT_sb[:, kt, q0:q0 + cw],
                                 start=(kt == 0), stop=(kt == KT - 1))
                yield
            nc.vector.tensor_scalar_add(dst[:, q0:q0 + cw], ps[:, :cw],
                                        b_sb[:, hp:hp + 1])
            yield

        def gen_opchain(hps, st, cc, kind):
            # out-proj contributions of head pairs `hps` (PSUM-accumulated),
            # folded into acc by one DVE add. kind: "bias" adds bo on the
            # first fold, "mid" accumulates, "final" folds in place and DMAs
            # straight from acc (a staging tile would serialize the tail on
            # its slot's DMA round-trip).
            sw = ST_SIZES[st]
            s0 = st * P
            c0 = cc * 512
            ps = psum.tile([P, 512], FP, tag="mm", bufs=2)
            for i, h in enumerate(hps):
                nc.tensor.matmul(ps[:sw, :],
                                 lhsT=ot_tiles[h][:, s0:s0 + sw],
                                 rhs=wo_tiles[h][:, c0:c0 + 512],
                                 start=(i == 0), stop=(i == len(hps) - 1))
                yield
            if kind == "bias":
                nc.vector.tensor_add(acc_sb[:sw, st, c0:c0 + 512],
                                     ps[:sw, :], bo_bc[:sw, c0:c0 + 512])
            else:
                nc.vector.tensor_add(acc_sb[:sw, st, c0:c0 + 512],
                                     acc_sb[:sw, st, c0:c0 + 512], ps[:sw, :])
                if kind == "final":
                    nc.sync.dma_start(out=out[s0:s0 + sw, c0:c0 + 512],
                                      in_=acc_sb[:sw, st, c0:c0 + 512])
            yield

        def gen_opfinal_st(st):
            # pairs 6+7 PSUM-accumulated per 512-col half, folded into acc,
            # each half DMA'd right after its fold so the last transfer in
            # the drain tail is a single 512-col block
            sw = ST_SIZES[st]
            s0 = st * P
            for cc in range(2):
                c0 = cc * 512
                ps = psum.tile([P, 512], FP, tag="mm", bufs=2, name="psf")
                for i, h in enumerate((6, 7)):
                    nc.tensor.matmul(ps[:sw, :],
                                     lhsT=ot_tiles[h][:, s0:s0 + sw],
                                     rhs=wo_tiles[h][:, c0:c0 + 512],
                                     start=(i == 0), stop=(i == 1))
                    yield
                nc.vector.tensor_add(acc_sb[:sw, st, c0:c0 + 512],
                                     acc_sb[:sw, st, c0:c0 + 512], ps[:sw, :])
                nc.sync.dma_start(out=out[s0:s0 + sw, c0:c0 + 512],
                                  in_=acc_sb[:sw, st, c0:c0 + 512])
                yield

        # Filler chains share one PSUM tag (bufs=2), so chain processing must
        # be strictly sequential — a single active chain at a time — or slot
        # rotation can order a matmul before the eviction it waits on.
        urgent = deque()
        lazy = deque()
        state = {"active": None}

        def _take():
            if urgent:
                return urgent.popleft()
            if lazy:
                return lazy.popleft()
            return None

        def pump(n):
            while n > 0:
                g = state["active"]
                if g is None:
                    g = _take()
                    if g is None:
                        return
                    state["active"] = g
                try:
                    next(g)
                    n -= 1
                except StopIteration:
                    state["active"] = None

        def _finish(g):
            for _ in g:
                pass

        def drain_urgent(max_lazy):
            # finish the active chain and all urgent chains; then trim the
            # lazy backlog so ot/acc consumers can't fall >1 pair behind
            g = state["active"]
            if g is not None:
                _finish(g)
                state["active"] = None
            while urgent:
                _finish(urgent.popleft())
            while len(lazy) > max_lazy:
                _finish(lazy.popleft())

        def drain_all():
            drain_urgent(0)

        for _rep in range(repeat):
            # ---- phase A: V quad 0 + Q/K projections for pair 0, emitted
            # interleaved to track the xT DMA arrival order
            qt_tiles[0] = qkp.tile([P, S], BF, tag="qt", name="qt0")
            kt_tiles[0] = qkp.tile([P, S], BF, tag="kt", name="kt0")
            phase_a = []
            for st in range(NST):
                phase_a.append(gen_vchain(0, st))
                if st % 2 == 1 and st < 9:
                    ci = st // 2
                    q0, cw = PROJ_CHUNKS[ci]
                    phase_a.append(gen_qkchain(0, "q", q0, cw))
                    phase_a.append(gen_qkchain(0, "k", q0, cw))
            q0, cw = PROJ_CHUNKS[4]
            phase_a.append(gen_qkchain(0, "q", q0, cw))
            phase_a.append(gen_qkchain(0, "k", q0, cw))
            for g in phase_a:
                _finish(g)

            # ---- steady loop over head pairs
            pending_op = []
            for hp in range(NPAIR):
                # prefetch weights two pairs ahead
                if hp + 2 < NPAIR:
                    dma_wqk(hp + 2)
                if hp in (1, 2):
                    dma_wv(hp + 1)
                # queue filler work: Q/K proj for hp+1 and V quads (urgent);
                # out-proj is pushed lazily per finished chunk
                if hp + 1 < NPAIR:
                    qt_tiles[hp + 1] = qkp.tile([P, S], BF, tag="qt",
                                                name=f"qt{hp + 1}")
                    kt_tiles[hp + 1] = qkp.tile([P, S], BF, tag="kt",
                                                name=f"kt{hp + 1}")
                    for (q0, cw) in PROJ_CHUNKS:
                        urgent.append(gen_qkchain(hp + 1, "q", q0, cw))
                        urgent.append(gen_qkchain(hp + 1, "k", q0, cw))
                quad = {0: 1, 2: 2, 3: 3}.get(hp)
                if quad is not None:
                    for st in range(NST):
                        urgent.append(gen_vchain(quad, st))

                qt_t, kt_t = qt_tiles[hp], kt_tiles[hp]
                ot_t = otp.tile([P, S], FPR, tag="ot", name=f"ot{hp}")
                ot_tiles[hp] = ot_t

                chunks = ATT_CHUNKS
                chunk_sts = CHUNK_STS
                last_ci = len(chunks) - 1
                for ci, (q0, cw) in enumerate(chunks):
                    deferred_op = pending_op
                    pending_op = []
                    oA = psum.tile([DH + 1, 512], FP, tag="oa", bufs=1)
                    oB = psum.tile([DH + 1, 512], FP, tag="ob", bufs=1)

                    def pv(eA, eB, kw, ks):
                        nc.tensor.matmul(
                            oA[:, :cw],
                            lhsT=v_sb[0:kw, ks, 2 * hp, :],
                            rhs=eA[:kw, :cw],
                            start=(ks == 0), stop=(ks == NST - 1))
                        nc.tensor.matmul(
                            oB[:, :cw],
                            lhsT=v_sb[0:kw, ks, 2 * hp + 1, :],
                            rhs=eB[:kw, :cw],
                            start=(ks == 0), stop=(ks == NST - 1))

                    pend = None
                    for ks in range(NST):
                        k0, kw = ks * P, ST_SIZES[ks]
                        sA = psum.tile([P, 512], FP, tag="sa", bufs=2)
                        sB = psum.tile([P, 512], FP, tag="sb", bufs=2)
                        nc.tensor.matmul(
                            sA[:kw, :cw],
                            lhsT=kt_t[0:DH, k0:k0 + kw],
                            rhs=qt_t[0:DH, q0:q0 + cw],
                            start=True, stop=True, tile_position=(0, 0))
                        nc.tensor.matmul(
                            sB[:kw, :cw],
                            lhsT=kt_t[DH:P, k0:k0 + kw],
                            rhs=qt_t[DH:P, q0:q0 + cw],
                            start=True, stop=True, tile_position=(64, 0))
                        eA = ep.tile([P, 512], BF, tag="ea")
                        eB = ep.tile([P, 512], BF, tag="eb")
                        nc.scalar.activation(eA[:kw, :cw], sA[:kw, :cw],
                                             AF.Exp, scale=float(SCALE))
                        nc.scalar.activation(eB[:kw, :cw], sB[:kw, :cw],
                                             AF.Exp, scale=float(SCALE))
                        if pend is not None:
                            pv(*pend)
                        pend = (eA, eB, kw, ks)
                        if ks == 4:
                            # out-proj chains deferred from the previous
                            # chunk: by now their OT columns (normalized via
                            # the DRAM Z-bounce) are resident, so pumping
                            # them can't stall the PE stream
                            lazy.extend(deferred_op)
                        pump(1)
                    pv(*pend)

                    # normalize: copy PV out to SBUF early (frees the PSUM
                    # banks for the next chunk), then bounce 1/Z through DRAM
                    # to replicate across partitions. The kernel-final chunk
                    # skips the copies (no next chunk) and reads PSUM.
                    final_chunk = (hp == NPAIR - 1 and ci == last_ci)
                    zt = zp.tile([P, 2, 512], FP, tag="zt")
                    if cw < 512 and not final_chunk:
                        # the bounce DMA reads whole rows; fill the
                        # never-written tail so stale-slot reads see
                        # defined data
                        nc.vector.memset(zt[DH:DH + 1, :, cw:512], 1.0)
                    if final_chunk:
                        nc.vector.reciprocal(zt[DH:DH + 1, 0, :cw],
                                             oA[DH:DH + 1, :cw])
                        nc.vector.reciprocal(zt[DH:DH + 1, 1, :cw],
                                             oB[DH:DH + 1, :cw])
                    else:
                        oA_sb = obp.tile([DH + 1, 512], FP, tag="oasb")
                        oB_sb = obp.tile([DH + 1, 512], FP, tag="obsb")
                        nc.vector.tensor_copy(oA_sb[:, :cw], oA[:, :cw])
                        nc.vector.tensor_copy(oB_sb[:, :cw], oB[:, :cw])
                        nc.vector.reciprocal(zt[DH:DH + 1, 0, :cw],
                                             oA_sb[DH:DH + 1, :cw])
                        nc.vector.reciprocal(zt[DH:DH + 1, 1, :cw],
                                             oB_sb[DH:DH + 1, :cw])
                    if final_chunk:
                        # last chunk of the kernel: the ~2us DRAM Z-bounce
                        # round-trip would sit fully in the drain tail, so
                        # broadcast 1/Z across partitions with two small PE
                        # matmuls (ones[64]^T @ zrow) into the just-freed
                        # oA/oB banks instead; the muls read the PV PSUM
                        # directly so the SBUF-copy hop leaves the critical
                        # path too
                        zbA = psum.tile([DH, 512], FP, tag="sa", bufs=2,
                                        name="zbA")
                        zbB = psum.tile([DH, 512], FP, tag="sb", bufs=2,
                                        name="zbB")
                        nc.tensor.matmul(zbA[:, :cw], lhsT=ones_sb[DH:DH + 1, :],
                                         rhs=zt[DH:DH + 1, 0, :cw],
                                         start=True, stop=True,
                                         tile_position=(64, 0))
                        nc.tensor.matmul(zbB[:, :cw], lhsT=ones_sb[DH:DH + 1, :],
                                         rhs=zt[DH:DH + 1, 1, :cw],
                                         start=True, stop=True,
                                         tile_position=(64, 0))
                        zbA_sb = obp.tile([DH, 512], FP, tag="oasb",
                                          name="zbAsb")
                        zbB_sb = obp.tile([DH, 512], FP, tag="obsb",
                                          name="zbBsb")
                        nc.vector.tensor_copy(zbA_sb[:, :cw], zbA[:, :cw])
                        nc.vector.tensor_copy(zbB_sb[:, :cw], zbB[:, :cw])
                        nc.vector.tensor_mul(ot_t[0:DH, q0:q0 + cw],
                                             oA[0:DH, :cw], zbA_sb[:, :cw])
                        otB = obp.tile([DH, 512], FPR, tag="otb", bufs=1,
                                       name="otBf")
                        nc.vector.tensor_mul(otB[:, :cw], oB[0:DH, :cw],
                                             zbB_sb[:, :cw])
                        nc.sync.dma_start(out=ot_t[DH:P, q0:q0 + cw],
                                          in_=otB[:, :cw])
                    else:
                        nc.sync.dma_start(out=zdram[hp, ci, :],
                                          in_=zt[DH:DH + 1, :, :])
                        zb = zp.tile([DH, 1024], FP, tag="zb")
                        zsrc = zdram[hp, ci, :]
                        nc.sync.dma_start(
                            out=zb,
                            in_=bass.AP(tensor=zsrc.tensor, offset=zsrc.offset,
                                        ap=[[0, DH], *zsrc.ap]))
                        # head A rows -> partitions 0-63 of ot
                        nc.vector.tensor_mul(ot_t[0:DH, q0:q0 + cw],
                                             oA_sb[0:DH, :cw], zb[:, 0:cw])
                        # head B rows must land at partitions 64-127; compute
                        # at base 0 then DMA-shift partitions
                        otB = obp.tile([DH, 512], FPR, tag="otb", bufs=1)
                        nc.vector.tensor_mul(otB[:, :cw], oB_sb[0:DH, :cw],
                                             zb[:, 512:512 + cw])
                        nc.sync.dma_start(out=ot_t[DH:P, q0:q0 + cw],
                                          in_=otB[:, :cw])
                    # out-projection for finished query cols: pairs
                    # (0,1),(2,3),(4,5) fold jointly; pair 7 folds alone
                    # (pair 6 was queued at this pair's start)
                    if hp == NPAIR - 1:
                        for st in chunk_sts[ci]:
                            pending_op.append(gen_opfinal_st(st))
                    elif hp % 2 == 1:
                        kind = "bias" if hp == 1 else "mid"
                        for st in chunk_sts[ci]:
                            for cc in range(2):
                                pending_op.append(
                                    gen_opchain((hp - 1, hp), st, cc, kind))
                    pump(8)
                # Q/K for hp+1 must be resident before its attention starts.
                # wo for hp+1 is DMA'd here (not earlier): its wait on the
                # previous wo slot's readers must sit after this pair's z/otB
                # DMAs in the SP stream, or the SP queue head deadlocks
                # against the PE stream.
                if hp + 1 < NPAIR:
                    dma_wo(hp + 1)
                drain_urgent(16)
            lazy.extend(pending_op)
            pending_op = []
            drain_all()
    _legalize_syncs(nc)
    return nc


_NC_CACHE = []


def _get_nc():
    if not _NC_CACHE:
        _NC_CACHE.append(build_nc())
    return _NC_CACHE[0]


def _in_maps(x, Wq, bq, Wk, bk, Wv, bv, Wo, bo):
    import ml_dtypes
    bf16 = ml_dtypes.bfloat16
    f = lambda a: np.asarray(a, dtype=np.float32)
    Wq, Wk, Wv, Wo = f(Wq), f(Wk), f(Wv), f(Wo)
    WqR = np.ascontiguousarray(
        Wq.reshape(KT, P, NPAIR, P).transpose(1, 2, 0, 3).astype(bf16))
    WkR = np.ascontiguousarray(
        Wk.reshape(KT, P, NPAIR, P).transpose(1, 2, 0, 3).astype(bf16))
    WvR = np.ascontiguousarray(
        Wv.reshape(KT, P, NQUAD, 256).transpose(1, 2, 0, 3).astype(bf16))
    WoR = np.ascontiguousarray(Wo.reshape(NPAIR, P, D).transpose(1, 0, 2))
    shared = {"WqR": WqR, "WkR": WkR, "WvR": WvR, "WoR": WoR,
              "bq": f(bq), "bk": f(bk), "bv": f(bv), "bo": f(bo)}
    x = f(x)
    maps = []
    for b in range(B):
        xT = np.zeros((D, SPAD), dtype=np.float32)
        xT[:, :S] = x[b].T
        xRb = np.ascontiguousarray(
            xT.reshape(KT, P, NXC, 256).transpose(1, 2, 0, 3).astype(bf16))
        maps.append({"xR": xRb, **shared})
    return maps


def kernel(x, Wq, bq, Wk, bk, Wv, bv, Wo, bo):
    nc = _get_nc()
    in_maps = _in_maps(x, Wq, bq, Wk, bk, Wv, bv, Wo, bo)
    res = run_bass_kernel_spmd(nc, in_maps, list(range(B)))
    return np.stack([res.results[b]["out"] for b in range(B)], axis=0)


# revision 44
# speedup vs baseline: 1.0210x; 1.0210x over previous
"""DinoV2 attention (B=8, S=1370, D=1024, H=16, Dh=64) on 8 trn2 NeuronCores.

Sharding: data parallel over batch — core b computes batch element b end to
end; weights are replicated; no collectives.

Cost-model-driven design (TimelineSim charges matmuls by output free-dim
rows only; Activation by free rows + fixed per-inst access penalty):
  - All projection matmuls use full 128-partition contraction (the out-proj
    is a single 128-deep matmul per tile, not two 64-deep tile_position
    halves, which would be charged double).
  - A software pipeline keeps the PE stream dense: while the Act-bound
    attention inner loop runs for pair hp, the PE stream is fed "filler"
    matmuls (Q/K projections for hp+1, V projection quads, out-projection
    accumulation for already-finished chunks) via an emission-order queue.
  - Out-projection accumulates two head pairs per PSUM tile, then folds the
    result into an SBUF accumulator via DVE adds; tiles are pushed as soon
    as their OT query-columns normalize, so the end-of-kernel tail is only
    the last chunk's tiles.
  - Host pre-swizzles x and the weights into partition-major bf16 layouts
    (fp32r only where precision matters: OT and Wo) so every DMA has
    contiguous >=512B runs per partition and arrives in the order the PE
    stream consumes it (256-column chunks for x, so the first V projection
    chain starts ~4us in while the rest of x still streams).
  - The kernel-final chunk normalizes via a small PE broadcast matmul
    (ones^T @ 1/Z) instead of the DRAM Z-bounce, and the final fold+store
    chains read/write the accumulator in place, keeping the drain tail to
    the DVE fold throughput.
Measured (per-core instruction-cost model): 372.1us vs 496.7us baseline;
hardware rel err 5.1e-3 (budget 2e-2). PE busy is ~355us, the cost-model
floor for this decomposition (charged matmul rows: QKVO projections
355.6k cycles + scores/PV 482.2k cycles); fp8 DoubleRow would halve the
score matmuls but inherently costs 4.4e-2 max-norm error (measured on the
reference) and is excluded by the 2e-2 gate.
"""

import numpy as np
from collections import deque
from contextlib import ExitStack

import concourse.bass as bass
import concourse.mybir as mybir
import concourse.tile as tile
from concourse.bass_utils import run_bass_kernel_spmd

B = 8
S = 1370
SPAD = 1536          # xT columns padded to 256-col DMA chunks (the pad
                     # columns are never read; 512B runs per partition dodge
                     # the <512B DMA descriptor penalty)
NXC = SPAD // 256    # 6 xT DMA chunks
D = 1024
H = 16
DH = 64
P = 128
KT = D // P          # 8 contraction tiles over D
NPAIR = H // 2       # 8 head pairs
NQUAD = 4            # V projection in 4-head (256-col) quads
NST = 11             # s-tiles over the real 1370 rows (last has 90)
FP = mybir.dt.float32
FPR = mybir.dt.float32r
BF = mybir.dt.bfloat16
AF = mybir.ActivationFunctionType

ST_SIZES = [min(P, S - i * P) for i in range(NST)]
# Q/K projection column chunks (padded space; 256-wide so each chunk only
# needs 2 xT s-tiles, letting projections start while xT still streams in)
PROJ_CHUNKS = [(0, 256), (256, 256), (512, 256), (768, 256), (1024, S - 1024)]
# attention query chunks (real queries only)
ATT_CHUNKS = [(0, 512), (512, 512), (1024, S - 1024)]
# s-tiles whose OT columns are final after each attention chunk
CHUNK_STS = [range(0, 4), range(4, 8), range(8, 11)]
SCALE = 1.0 / np.sqrt(DH)


def _legalize_syncs(nc):
    """Move excess sem waits onto injected NoOps.

    This walrus build encodes at most one wait (plus one update) per TPB
    instruction; Tile emits several. Engines execute their streams in
    order and the Tile schedule is a topological order of the dependency
    DAG, so hoisting waits onto preceding same-engine NoOps preserves
    progress (anything scheduled earlier can still complete) and
    correctness (the instruction still starts only after all its waits).
    """
    nid = 0
    for f in nc.m.functions:
        for blk in f.blocks:
            out = []
            for inst in blk.instructions:
                si = inst.sync_info
                if si is not None:
                    waits = list(si.on_wait)
                    ups = list(si.on_update)
                    if len(waits) > 1:
                        for w in waits[:-1]:
                            nop = mybir.InstNoOp(
                                name=f"I-syncfix-{nid}",
                                engine=inst.engine, ins=[], outs=[],
                                sync_info=mybir.SyncInfo(on_wait=[w],
                                                         on_update=[]))
                            nid += 1
                            nc.register_instruction(nop)
                            out.append(nop)
                        inst.sync_info = mybir.SyncInfo(on_wait=waits[-1:],
                                                        on_update=ups)
                out.append(inst)
            blk.instructions = out


def build_nc(repeat=1):
    nc = bass.Bass()
    # host-swizzled inputs (see _in_maps)
    xR = nc.declare_dram_parameter("xR", [P, NXC, KT, 256], BF, isOutput=False)
    WqR = nc.declare_dram_parameter("WqR", [P, NPAIR, KT, P], BF,
                                    isOutput=False)
    WkR = nc.declare_dram_parameter("WkR", [P, NPAIR, KT, P], BF,
                                    isOutput=False)
    WvR = nc.declare_dram_parameter("WvR", [P, NQUAD, KT, 256], BF,
                                    isOutput=False)
    WoR = nc.declare_dram_parameter("WoR", [P, NPAIR, D], BF, isOutput=False)
    idn = nc.declare_dram_parameter("idn", [P, P], BF, isOutput=False)
    bq = nc.declare_dram_parameter("bq", [D], FP, isOutput=False)
    bk = nc.declare_dram_parameter("bk", [D], FP, isOutput=False)
    bv = nc.declare_dram_parameter("bv", [D], FP, isOutput=False)
    bo = nc.declare_dram_parameter("bo", [D], FP, isOutput=False)
    out = nc.declare_dram_parameter("out", [S, D], FP, isOutput=True)
    def bcast(handle, rows):
        a = handle[:]
        return bass.AP(tensor=a.tensor, offset=a.offset, ap=[[0, rows], *a.ap])

    xR_ap = xR[:]
    wq_ap = WqR[:].rearrange("p hp kt i -> p (hp kt) i")
    wk_ap = WkR[:].rearrange("p hp kt i -> p (hp kt) i")
    wv_ap = WvR[:].rearrange("p q kt j -> p (q kt) j")
    wo_ap = WoR[:].rearrange("p hp c -> p (hp c)")

    with ExitStack() as ctx:
        tc = ctx.enter_context(tile.TileContext(nc))
        const = ctx.enter_context(tc.tile_pool(name="const", bufs=1))
        persist = ctx.enter_context(tc.tile_pool(name="persist", bufs=1))
        psum = ctx.enter_context(tc.tile_pool(name="psum", bufs=1,
                                              space="PSUM"))
        wp = ctx.enter_context(tc.tile_pool(name="wp", bufs=2))
        wvp = ctx.enter_context(tc.tile_pool(name="wvp", bufs=2))
        wop = ctx.enter_context(tc.tile_pool(name="wop", bufs=2))
        qkp = ctx.enter_context(tc.tile_pool(name="qkp", bufs=2))
        otp = ctx.enter_context(tc.tile_pool(name="otp", bufs=3))
        ep = ctx.enter_context(tc.tile_pool(name="ep", bufs=2))
        zp = ctx.enter_context(tc.tile_pool(name="zp", bufs=1))
        obp = ctx.enter_context(tc.tile_pool(name="obp", bufs=2))

        # persistent state
        xT_sb = persist.tile([P, KT, SPAD], BF)        # x^T, kt-major
        v_sb = persist.tile([P, NST, H, DH + 1], BF)    # V + ones column
        acc_sb = persist.tile([P, NST, D], FP)          # out-proj accumulator

        idn_sb = const.tile([P, P], BF)
        bq_sb = const.tile([P, NPAIR], FP)
        bk_sb = const.tile([P, NPAIR], FP)
        bv_bc = const.tile([P, D], FP)
        bo_bc = const.tile([P, D], FP)

        wv_tiles = {}
        wq_tiles, wk_tiles = {}, {}
        wo_tiles = {}
        qt_tiles, kt_tiles = {}, {}
        ot_tiles = {}

        def dma_wv(q):
            wv_tiles[q] = wvp.tile([P, KT, 256], BF, tag="wv", name=f"wv{q}")
            nc.sync.dma_start(out=wv_tiles[q],
                              in_=wv_ap[:, q * KT:(q + 1) * KT, :])

        def dma_wqk(hp):
            wq_tiles[hp] = wp.tile([P, KT, P], BF, tag="wq", name=f"wq{hp}")
            wk_tiles[hp] = wp.tile([P, KT, P], BF, tag="wk", name=f"wk{hp}")
            nc.sync.dma_start(out=wq_tiles[hp],
                              in_=wq_ap[:, hp * KT:(hp + 1) * KT, :])
            nc.sync.dma_start(out=wk_tiles[hp],
                              in_=wk_ap[:, hp * KT:(hp + 1) * KT, :])

        def dma_wo(hp):
            wo_tiles[hp] = wop.tile([P, D], BF, tag="wo", name=f"wo{hp}")
            nc.sync.dma_start(out=wo_tiles[hp],
                              in_=wo_ap[:, hp * D:(hp + 1) * D])

        # ---- startup DMA stream (arrival order == emission order): the
        # first V chain needs only wv quad 0 kt 0-3 + xT s-tile 0, so those
        # go out first in small pieces
        wv_tiles[0] = wvp.tile([P, KT, 256], BF, tag="wv", name="wv0")
        nc.sync.dma_start(out=wv_tiles[0][:, 0:4, :],
                          in_=wv_ap[:, 0:4, :])
        nc.sync.dma_start(out=xT_sb[:, 0:4, 0:256], in_=xR_ap[:, 0, 0:4])
        nc.sync.dma_start(out=xT_sb[:, 4:8, 0:256], in_=xR_ap[:, 0, 4:8])
        nc.sync.dma_start(out=wv_tiles[0][:, 4:8, :],
                          in_=wv_ap[:, 4:8, :])
        nc.sync.dma_start(out=xT_sb[:, :, 256:512], in_=xR_ap[:, 1])
        nc.sync.dma_start(out=bv_bc, in_=bcast(bv, P))
        dma_wqk(0)
        nc.sync.dma_start(out=bq_sb,
                          in_=bq[:].rearrange("(hp p) -> p hp", p=P))
        nc.sync.dma_start(out=bk_sb,
                          in_=bk[:].rearrange("(hp p) -> p hp", p=P))
        for xc in range(2, NXC):
            nc.sync.dma_start(out=xT_sb[:, :, xc * 256:(xc + 1) * 256],
                              in_=xR_ap[:, xc])
        nc.sync.dma_start(out=bo_bc, in_=bcast(bo, P))
        nc.sync.dma_start(out=idn_sb, in_=idn[:])
        dma_wv(1)
        dma_wqk(1)
        dma_wo(0)

        # ones columns for the fused softmax denominators (Z row of PV out)
        for st in range(NST):
            nc.vector.memset(v_sb[:, st, :, DH:DH + 1], 1.0)

        # ---- filler chain generators (one PE/DVE instruction per yield)
        def gen_vchain(quad, st):
            sw = ST_SIZES[st]
            wv_t = wv_tiles[quad]
            vps = psum.tile([P, 512], FP, tag="mm", bufs=2)
            for kt in range(KT):
                nc.tensor.matmul(vps[:sw, :256],
                                 lhsT=xT_sb[:, kt, st * P:st * P + sw],
                                 rhs=wv_t[:, kt, :],
                                 start=(kt == 0), stop=(kt == KT - 1))
                yield
            h0 = quad * 4
            c0 = quad * 256
            nc.vector.tensor_add(
                v_sb[:sw, st, h0:h0 + 4, 0:DH],
                vps[:sw, :256].rearrange("p (h d) -> p h d", h=4),
                bv_bc[:sw, c0:c0 + 256].rearrange("p (h d) -> p h d", h=4))
            yield

        def gen_qkchain(hp, which, q0, cw):
            if which == "q":
                w_t, dst, b_sb = wq_tiles[hp], qt_tiles[hp], bq_sb
            else:
                w_t, dst, b_sb = wk_tiles[hp], kt_tiles[hp], bk_sb
            ps = psum.tile([P, 512], FP, tag="mm", bufs=2)
            for kt in range(KT):
                nc.tensor.matmul(ps[:, :cw],
                                 lhsT=w_t[:, kt, :],
                                 rhs=xT_sb[:, kt, q0:q0 + cw],
                                 start=(kt == 0), stop=(kt == KT - 1))
                yield
            nc.vector.tensor_scalar_add(dst[:, q0:q0 + cw], ps[:, :cw],
                                        b_sb[:, hp:hp + 1])
            yield

        def gen_opchain(hps, st, cc, kind):
            # out-proj contributions of head pairs `hps` (PSUM-accumulated),
            # folded into acc by one DVE add. kind: "bias" adds bo on the
            # first fold, "mid" accumulates, "final" folds in place and DMAs
            # straight from acc (a staging tile would serialize the tail on
            # its slot's DMA round-trip).
            sw = ST_SIZES[st]
            s0 = st * P
            c0 = cc * 512
            ps = psum.tile([P, 512], FP, tag="mm", bufs=2)
            for i, h in enumerate(hps):
                nc.tensor.matmul(ps[:sw, :],
                                 lhsT=ot_tiles[h][:, s0:s0 + sw],
                                 rhs=wo_tiles[h][:, c0:c0 + 512],
                                 start=(i == 0), stop=(i == len(hps) - 1))
                yield
            if kind == "bias":
                nc.vector.tensor_add(acc_sb[:sw, st, c0:c0 + 512],
                                     ps[:sw, :], bo_bc[:sw, c0:c0 + 512])
            else:
                nc.vector.tensor_add(acc_sb[:sw, st, c0:c0 + 512],
                                     acc_sb[:sw, st, c0:c0 + 512], ps[:sw, :])
                if kind == "final":
                    nc.sync.dma_start(out=out[s0:s0 + sw, c0:c0 + 512],
                                      in_=acc_sb[:sw, st, c0:c0 + 512])
            yield

        def gen_opfinal_st(st):
            # pairs 6+7 PSUM-accumulated per 512-col half, folded into acc,
            # each half DMA'd right after its fold so the last transfer in
            # the drain tail is a single 512-col block
            sw = ST_SIZES[st]
            s0 = st * P
            for cc in range(2):
                c0 = cc * 512
                ps = psum.tile([P, 512], FP, tag="mm", bufs=2, name="psf")
                for i, h in enumerate((6, 7)):
                    nc.tensor.matmul(ps[:sw, :],
                                     lhsT=ot_tiles[h][:, s0:s0 + sw],
                                     rhs=wo_tiles[h][:, c0:c0 + 512],
                                     start=(i == 0), stop=(i == 1))
                    yield
                nc.vector.tensor_add(acc_sb[:sw, st, c0:c0 + 512],
                                     acc_sb[:sw, st, c0:c0 + 512], ps[:sw, :])
                nc.sync.dma_start(out=out[s0:s0 + sw, c0:c0 + 512],
                                  in_=acc_sb[:sw, st, c0:c0 + 512])
                yield

        def gen_tp(otq, ot_t, q0, subs):
            # transpose O[q, d] back to OT[d, q] on the PE; head B lands at
            # partitions 64-127 via tile_position
            tp = psum.tile([P, 4, P], BF, tag="tp", bufs=1, name="tp")
            for sub, (q1, w) in enumerate(subs):
                nc.tensor.transpose(tp[0:DH, sub, :w], otq[:w, 0, sub, :],
                                    idn_sb[:w, :w], tile_position=(0, 0))
                yield
                nc.tensor.transpose(tp[DH:P, sub, :w], otq[:w, 1, sub, :],
                                    idn_sb[:w, :w], tile_position=(0, 64))
                yield
            for sub, (q1, w) in enumerate(subs):
                nc.vector.tensor_copy(ot_t[:, q0 + q1:q0 + q1 + w],
                                      tp[:, sub, :w])
                yield

        # Filler chains share one PSUM tag (bufs=2), so chain processing must
        # be strictly sequential — a single active chain at a time — or slot
        # rotation can order a matmul before the eviction it waits on.
        urgent = deque()
        lazy = deque()
        tp_q = deque()
        state = {"active": None}

        def _take():
            if urgent:
                return urgent.popleft()
            if tp_q:
                return tp_q.popleft()
            if lazy:
                return lazy.popleft()
            return None

        def pump(n):
            while n > 0:
                g = state["active"]
                if g is None:
                    g = _take()
                    if g is None:
                        return
                    state["active"] = g
                try:
                    next(g)
                    n -= 1
                except StopIteration:
                    state["active"] = None

        def _finish(g):
            for _ in g:
                pass

        def drain_tp(max_left):
            # transpose chains gate the chunk-end DVE ops via the otq slot;
            # older chains must be fully emitted before a new normalize
            if len(tp_q) <= max_left:
                return
            g = state["active"]
            if g is not None:
                _finish(g)
                state["active"] = None
            while len(tp_q) > max_left:
                _finish(tp_q.popleft())

        def drain_urgent(max_lazy):
            # finish the active chain and all urgent chains; then trim the
            # lazy backlog so ot/acc consumers can't fall >1 pair behind
            g = state["active"]
            if g is not None:
                _finish(g)
                state["active"] = None
            while tp_q:
                _finish(tp_q.popleft())
            while urgent:
                _finish(urgent.popleft())
            while len(lazy) > max_lazy:
                _finish(lazy.popleft())

        def drain_all():
            drain_urgent(0)

        for _rep in range(repeat):
            # ---- phase A: V quad 0 + Q/K projections for pair 0, emitted
            # interleaved to track the xT DMA arrival order
            qt_tiles[0] = qkp.tile([P, S], BF, tag="qt", name="qt0")
            kt_tiles[0] = qkp.tile([P, S], BF, tag="kt", name="kt0")
            phase_a = []
            for st in range(NST):
                phase_a.append(gen_vchain(0, st))
                if st % 2 == 1 and st < 9:
                    ci = st // 2
                    q0, cw = PROJ_CHUNKS[ci]
                    phase_a.append(gen_qkchain(0, "q", q0, cw))
                    phase_a.append(gen_qkchain(0, "k", q0, cw))
            q0, cw = PROJ_CHUNKS[4]
            phase_a.append(gen_qkchain(0, "q", q0, cw))
            phase_a.append(gen_qkchain(0, "k", q0, cw))
            for g in phase_a:
                _finish(g)

            # ---- steady loop over head pairs
            pending_op = []
            for hp in range(NPAIR):
                # prefetch weights two pairs ahead
                if hp + 2 < NPAIR:
                    dma_wqk(hp + 2)
                if hp in (1, 2):
                    dma_wv(hp + 1)
                # queue filler work: Q/K proj for hp+1 and V quads (urgent);
                # out-proj is pushed lazily per finished chunk
                if hp + 1 < NPAIR:
                    qt_tiles[hp + 1] = qkp.tile([P, S], BF, tag="qt",
                                                name=f"qt{hp + 1}")
                    kt_tiles[hp + 1] = qkp.tile([P, S], BF, tag="kt",
                                                name=f"kt{hp + 1}")
                    for (q0, cw) in PROJ_CHUNKS:
                        urgent.append(gen_qkchain(hp + 1, "q", q0, cw))
                        urgent.append(gen_qkchain(hp + 1, "k", q0, cw))
                quad = {0: 1, 2: 2, 3: 3}.get(hp)
                if quad is not None:
                    for st in range(NST):
                        urgent.append(gen_vchain(quad, st))

                qt_t, kt_t = qt_tiles[hp], kt_tiles[hp]
                ot_t = otp.tile([P, S], BF, tag="ot", name=f"ot{hp}")
                ot_tiles[hp] = ot_t

                chunks = ATT_CHUNKS
                chunk_sts = CHUNK_STS
                last_ci = len(chunks) - 1
                for ci, (q0, cw) in enumerate(chunks):
                    deferred_op = pending_op
                    pending_op = []
                    drain_tp(1)
                    nsub = (cw + P - 1) // P
                    subs = [(s * P, min(P, cw - s * P)) for s in range(nsub)]
                    # transposed PV: O[q, d] with queries on partitions —
                    # the matmul moves only 65 dim-rows per accumulation
                    # step, and Z lands as a per-partition column. start=True
                    # clears has_written for the WHOLE bank, so only sub 0
                    # starts; the other subs' first writes overwrite via the
                    # cleared bits.
                    pvA = psum.tile([P, 4, P], FP, tag="oa", bufs=1)
                    pvB = psum.tile([P, 4, P], FP, tag="ob", bufs=1)

                    def pv(eA, eB, kw, ks):
                        for sub, (q1, w) in enumerate(subs):
                            st0 = (ks == 0 and sub == 0)
                            nc.tensor.matmul(
                                pvA[:w, sub, 0:DH + 1],
                                lhsT=eA[:kw, q1:q1 + w],
                                rhs=v_sb[0:kw, ks, 2 * hp, :],
                                start=st0, stop=(ks == NST - 1),
                                skip_group_check=True)
                            nc.tensor.matmul(
                                pvB[:w, sub, 0:DH + 1],
                                lhsT=eB[:kw, q1:q1 + w],
                                rhs=v_sb[0:kw, ks, 2 * hp + 1, :],
                                start=st0, stop=(ks == NST - 1),
                                skip_group_check=True)

                    pend = None
                    for ks in range(NST):
                        k0, kw = ks * P, ST_SIZES[ks]
                        sA = psum.tile([P, 512], FP, tag="sa", bufs=2)
                        sB = psum.tile([P, 512], FP, tag="sb", bufs=1)
                        nc.tensor.matmul(
                            sA[:kw, :cw],
                            lhsT=kt_t[0:DH, k0:k0 + kw],
                            rhs=qt_t[0:DH, q0:q0 + cw],
                            start=True, stop=True, tile_position=(0, 0))
                        nc.tensor.matmul(
                            sB[:kw, :cw],
                            lhsT=kt_t[DH:P, k0:k0 + kw],
                            rhs=qt_t[DH:P, q0:q0 + cw],
                            start=True, stop=True, tile_position=(64, 0))
                        eA = ep.tile([P, 512], BF, tag="ea")
                        eB = ep.tile([P, 512], BF, tag="eb")
                        nc.scalar.activation(eA[:kw, :cw], sA[:kw, :cw],
                                             AF.Exp, scale=float(SCALE))
                        nc.scalar.activation(eB[:kw, :cw], sB[:kw, :cw],
                                             AF.Exp, scale=float(SCALE))
                        if pend is not None:
                            pv(*pend)
                        pend = (eA, eB, kw, ks)
                        if ks == 4:
                            # out-proj chains deferred from the previous
                            # chunk: by now their OT columns (normalized via
                            # the DRAM Z-bounce) are resident, so pumping
                            # them can't stall the PE stream
                            lazy.extend(deferred_op)
                        pump(1)
                    pv(*pend)

                    # normalize: Z is the ones-column (free index 64) of the
                    # transposed PV output — a per-partition scalar, no
                    # cross-partition broadcast or DRAM bounce needed
                    zr = zp.tile([P, 2, 4, 1], FP, tag="zr")
                    nc.vector.reciprocal(zr[:, 0, :nsub, :],
                                         pvA[:, :nsub, DH:DH + 1])
                    nc.vector.reciprocal(zr[:, 1, :nsub, :],
                                         pvB[:, :nsub, DH:DH + 1])
                    otq = obp.tile([P, 2, 4, DH], BF, tag="otq")
                    for sub, (q1, w) in enumerate(subs):
                        nc.vector.tensor_scalar_mul(otq[:w, 0, sub, :],
                                                    pvA[:w, sub, 0:DH],
                                                    zr[:w, 0, sub, :])
                        nc.vector.tensor_scalar_mul(otq[:w, 1, sub, :],
                                                    pvB[:w, sub, 0:DH],
                                                    zr[:w, 1, sub, :])
                    tp_q.append(gen_tp(otq, ot_t, q0, subs))
                    # out-projection for finished query cols: pairs
                    # (0,1),(2,3),(4,5) fold jointly; pair 7 folds alone
                    # (pair 6 was queued at this pair's start)
                    if hp == NPAIR - 1:
                        for st in chunk_sts[ci]:
                            pending_op.append(gen_opfinal_st(st))
                    elif hp % 2 == 1:
                        kind = "bias" if hp == 1 else "mid"
                        for st in chunk_sts[ci]:
                            for cc in range(2):
                                pending_op.append(
                                    gen_opchain((hp - 1, hp), st, cc, kind))
                    pump(8)
                # Q/K for hp+1 must be resident before its attention starts.
                # wo for hp+1 is DMA'd here (not earlier): its wait on the
                # previous wo slot's readers must sit after this pair's z/otB
                # DMAs in the SP stream, or the SP queue head deadlocks
                # against the PE stream.
                if hp + 1 < NPAIR:
                    dma_wo(hp + 1)
                drain_urgent(6)
            lazy.extend(pending_op)
            pending_op = []
            drain_all()
    _legalize_syncs(nc)
    return nc


_NC_CACHE = []


def _get_nc():
    if not _NC_CACHE:
        _NC_CACHE.append(build_nc())
    return _NC_CACHE[0]


def _in_maps(x, Wq, bq, Wk, bk, Wv, bv, Wo, bo):
    import ml_dtypes
    bf16 = ml_dtypes.bfloat16
    f = lambda a: np.asarray(a, dtype=np.float32)
    Wq, Wk, Wv, Wo = f(Wq), f(Wk), f(Wv), f(Wo)
    WqR = np.ascontiguousarray(
        Wq.reshape(KT, P, NPAIR, P).transpose(1, 2, 0, 3).astype(bf16))
    WkR = np.ascontiguousarray(
        Wk.reshape(KT, P, NPAIR, P).transpose(1, 2, 0, 3).astype(bf16))
    WvR = np.ascontiguousarray(
        Wv.reshape(KT, P, NQUAD, 256).transpose(1, 2, 0, 3).astype(bf16))
    WoR = np.ascontiguousarray(
        Wo.reshape(NPAIR, P, D).transpose(1, 0, 2).astype(bf16))
    shared = {"WqR": WqR, "WkR": WkR, "WvR": WvR, "WoR": WoR,
              "idn": np.eye(P, dtype=bf16),
              "bq": f(bq), "bk": f(bk), "bv": f(bv), "bo": f(bo)}
    x = f(x)
    maps = []
    for b in range(B):
        xT = np.zeros((D, SPAD), dtype=np.float32)
        xT[:, :S] = x[b].T
        xRb = np.ascontiguousarray(
            xT.reshape(KT, P, NXC, 256).transpose(1, 2, 0, 3).astype(bf16))
        maps.append({"xR": xRb, **shared})
    return maps


def kernel(x, Wq, bq, Wk, bk, Wv, bv, Wo, bo):
    nc = _get_nc()
    in_maps = _in_maps(x, Wq, bq, Wk, bk, Wv, bv, Wo, bo)
    res = run_bass_kernel_spmd(nc, in_maps, list(range(B)))
    return np.stack([res.results[b]["out"] for b in range(B)], axis=0)
